# revision 8
# baseline (speedup 1.0000x reference)
"""Multi-head self-attention (B=2, T=2048, C=1024, H=16) on 8 NeuronCores.

Sharding: core c -> (batch b = c//4, head-group g = c%4); each core computes
4 heads' attention for one batch plus its slice of the QKV/out projections.
Per-core partial outputs (over head groups) are summed on the host.

Device-side layout is fully transposed (feature dim on partitions):
  xt [C, T] -> QT/KT [256, T] (j on partitions), V natural [T, 256],
  ST = K Qt (scores transposed, tk on partitions).
The stationary PV operand is V extended with 64 columns of ones, so the
yext accumulator's rows 64..127 all hold the softmax denominator.

v2 schedule (single fused pipeline, derived from the v1 trace):
 - xt DMA is tq-block-major so the first QK psum tile only waits ~1MB.
 - QK-jt0 runs first; then V-proj + QK-jt1 are woven between the
   scores+exp of three (head, qblock) attention units so the Scalar
   engine starts exp work ~40us earlier than v1.
 - Attention PV is software-pipelined at lag 2 behind scores/exp so the
   in-order PE never blocks on the exp->PV dependency (v1 lost ~130us
   of PE waits to this).
 - The out projection for query-block qb is issued right after the four
   units of that qb, overlapping its PSUM eviction and output DMA with
   the next qb's attention. Output is fp16 (host sums partials in fp32).
"""

import numpy as np

import concourse.bacc as bacc
import concourse.mybir as mybir
import concourse.tile as tile
from concourse.bass_utils import run_bass_kernel_spmd

B, T, C, H = 2, 2048, 1024, 16
HD = C // H  # 64
NCORES = 8
GROUPS = 4  # head groups (one per core within a batch)
HPG = H // GROUPS  # heads per group = 4
JW = HPG * HD  # per-core projection slice width = 256

F32 = mybir.dt.float32
F16 = mybir.dt.float16
MMDT = mybir.dt.float16
NPDT = np.float16

_CACHED_NC = None


def _build():
    nc = bacc.Bacc("TRN2", target_bir_lowering=False, num_devices=NCORES)

    xt = nc.dram_tensor("xt", [C, T], MMDT, kind="ExternalInput")
    wq = nc.dram_tensor("wq", [C, JW], MMDT, kind="ExternalInput")
    wk = nc.dram_tensor("wk", [C, JW], MMDT, kind="ExternalInput")
    wv = nc.dram_tensor("wv", [C, JW], MMDT, kind="ExternalInput")
    wo = nc.dram_tensor("wo", [JW, C], MMDT, kind="ExternalInput")
    bq = nc.dram_tensor("bq", [JW], F32, kind="ExternalInput")
    bk = nc.dram_tensor("bk", [JW], F32, kind="ExternalInput")
    out = nc.dram_tensor("out", [T, C], F16, kind="ExternalOutput")

    xt_ap = xt[:, :].rearrange("(cc p) t -> p cc t", p=128)  # [128, 8, T]
    wq_ap = wq[:, :].rearrange("(cc p) j -> p cc j", p=128)  # [128, 8, 256]
    wk_ap = wk[:, :].rearrange("(cc p) j -> p cc j", p=128)
    wv_ap = wv[:, :].rearrange("(cc p) j -> p cc j", p=128)
    wo_ap = wo[:, :].rearrange("(jt p) m -> p jt m", p=128)  # [128, 2, C]
    bq_ap = bq[:].rearrange("(jt p) -> p jt", p=128)  # [128, 2]
    bk_ap = bk[:].rearrange("(jt p) -> p jt", p=128)

    with tile.TileContext(nc) as tc:
        with (
            tc.tile_pool(name="big", bufs=1) as big,
            tc.tile_pool(name="es", bufs=46) as esp,
            tc.tile_pool(name="nrm", bufs=4) as nrm,
            tc.tile_pool(name="outp", bufs=3) as outp,
            tc.tile_pool(name="psA", bufs=2, space="PSUM") as psA,
            tc.tile_pool(name="psY", bufs=2, space="PSUM") as psY,
            tc.tile_pool(name="psO", bufs=1, space="PSUM") as psO,
        ):
            # ---- persistent SBUF tensors ----
            xt_sb = big.tile([128, 8, T], MMDT)
            wq_sb = big.tile([128, 8, JW], MMDT)
            wk_sb = big.tile([128, 8, JW], MMDT)
            wv_sb = big.tile([128, 8, JW], MMDT)
            wo_sb = big.tile([128, 2, C], MMDT)
            qt_sb = big.tile([128, 2, T], MMDT)
            kt_sb = big.tile([128, 2, T], MMDT)
            yt_sb = big.tile([128, 2, T], MMDT)
            # V natural + 64 ones columns per head (denominator broadcast rows)
            v_sb = big.tile([128, 16, HPG, 128], MMDT)
            bq_sb = big.tile([128, 2], F32)
            bk_sb = big.tile([128, 2], F32)

            # ---- DMA: order by first use; xt is tq-block-major so the
            # first QK psum tile needs only the first 8 pieces.
            nc.sync.dma_start(out=bq_sb[:], in_=bq_ap)
            nc.sync.dma_start(out=bk_sb[:], in_=bk_ap)
            nc.sync.dma_start(out=wq_sb[:], in_=wq_ap)
            nc.sync.dma_start(out=wk_sb[:], in_=wk_ap)
            nc.vector.memset(v_sb[:, :, :, HD:128], 1.0)
            for tqh in range(4):
                ts = slice(tqh * 512, (tqh + 1) * 512)
                for cc in range(8):
                    nc.sync.dma_start(out=xt_sb[:, cc, ts], in_=xt_ap[:, cc, ts])
                if tqh == 1:
                    nc.sync.dma_start(out=wv_sb[:], in_=wv_ap)
            nc.sync.dma_start(out=wo_sb[:], in_=wo_ap)

            # ---- emit helpers (each op is a closure; order = issue order) ----

            def qk_half(jt, th):
                """K then Q projection for 512-col tq block th (0..3), one
                [128,1024] psum tile: cols 0:512 = K (bank 1), 512:1024 = Q
                (bank 2). K is evicted first so scores can start early."""
                pkq = psA.tile([128, 1024], F32, tag="mm", name="pkq")
                xs = slice(th * 512, (th + 1) * 512)
                for cc in range(8):
                    nc.tensor.matmul(
                        pkq[:, 0:512],
                        wk_sb[:, cc, jt * 128 : (jt + 1) * 128],
                        xt_sb[:, cc, xs],
                        start=(cc == 0),
                        stop=(cc == 7),
                    )
                nc.vector.tensor_scalar_add(
                    out=kt_sb[:, jt, xs], in0=pkq[:, 0:512], scalar1=bk_sb[:, jt : jt + 1]
                )
                for cc in range(8):
                    nc.tensor.matmul(
                        pkq[:, 512:1024],
                        wq_sb[:, cc, jt * 128 : (jt + 1) * 128],
                        xt_sb[:, cc, xs],
                        start=(cc == 0),
                        stop=(cc == 7),
                    )
                nc.vector.tensor_scalar_add(
                    out=qt_sb[:, jt, xs], in0=pkq[:, 512:1024], scalar1=bq_sb[:, jt : jt + 1]
                )

            def v_tile(tg):
                """V projection for t-chunks 2*tg, 2*tg+1."""
                pv = psA.tile([128, 1024], F32, tag="mm", name="pv")
                for half in range(2):
                    tt = tg * 2 + half
                    for cc in range(8):
                        nc.tensor.matmul(
                            pv[:, half * 512 : half * 512 + JW],
                            xt_sb[:, cc, tt * 128 : (tt + 1) * 128],
                            wv_sb[:, cc, :],
                            start=(cc == 0),
                            stop=(cc == 7),
                        )
                pv3 = pv[:].rearrange("p (half j) -> p half j", half=2)
                nc.vector.tensor_copy(
                    out=v_sb[:, tg * 2 : tg * 2 + 2, :, 0:HD],
                    in_=pv3[:, :, 0:JW].rearrange("p half (h d) -> p half h d", h=HPG),
                )

            def score_step(h, qb, kg, es_list):
                """Scores + exp for kc pair (2kg, 2kg+1); appends the es tile."""
                jt, pb = h // 2, 64 * (h % 2)
                qs = slice(qb * 512, (qb + 1) * 512)
                st = psA.tile([128, 1024], F32, tag="mm", name="st")
                for half in range(2):
                    kc = kg * 2 + half
                    nc.tensor.matmul(
                        st[:, half * 512 : (half + 1) * 512],
                        kt_sb[pb : pb + HD, jt, kc * 128 : (kc + 1) * 128],
                        qt_sb[pb : pb + HD, jt, qs],
                        start=True,
                        stop=True,
                    )
                es = esp.tile([128, 1024], MMDT, tag="es", name="es")
                nc.scalar.activation(
                    out=es[:], in_=st[:], func=mybir.ActivationFunctionType.Exp
                )
                es_list.append(es)

            class Unit:
                """Attention unit (h, qb): 8 score/exp steps + 8 PV steps."""

                def __init__(self, h, qb):
                    self.h, self.qb = h, qb
                    self.es = []
                    self.yext = None
                    self.n_pv = 0

                def emit_score(self, kg):
                    score_step(self.h, self.qb, kg, self.es)

                def emit_pv(self):
                    kg = self.n_pv
                    if kg == 0:
                        self.yext = psY.tile([128, 512], F32, tag="yext", name="yext")
                    es = self.es[kg]
                    for half in range(2):
                        kc = kg * 2 + half
                        nc.tensor.matmul(
                            self.yext[:],
                            v_sb[:, kc, self.h, :],
                            es[:, half * 512 : (half + 1) * 512],
                            start=(kc == 0),
                            stop=(kc == 15),
                        )
                    self.n_pv += 1
                    if self.n_pv == 8:
                        self._normalize()

                def _normalize(self):
                    jt, pb = self.h // 2, 64 * (self.h % 2)
                    qs = slice(self.qb * 512, (self.qb + 1) * 512)
                    r32 = nrm.tile([HD, 512], F32, tag="r32", name="r32")
                    nc.vector.reciprocal(out=r32[:], in_=self.yext[HD:128, :])
                    nc.vector.tensor_mul(
                        out=yt_sb[pb : pb + HD, jt, qs],
                        in0=r32[:],
                        in1=self.yext[0:HD, :],
                    )

            def outproj(qb):
                for tt in range(qb * 4, (qb + 1) * 4):
                    po = psO.tile([128, 1024], F32, tag="po", name="po")
                    for jt in range(2):
                        for mb in range(2):
                            nc.tensor.matmul(
                                po[:, mb * 512 : (mb + 1) * 512],
                                yt_sb[:, jt, tt * 128 : (tt + 1) * 128],
                                wo_sb[:, jt, mb * 512 : (mb + 1) * 512],
                                start=(jt == 0),
                                stop=(jt == 1),
                            )
                    ob = outp.tile([128, 1024], F16, tag="ob", name="ob")
                    nc.vector.tensor_copy(out=ob[:], in_=po[:])
                    nc.sync.dma_start(out=out[tt * 128 : (tt + 1) * 128, :], in_=ob[:])

            # ---- phase A: QK jt0 woven with the first unit's scores so the
            # Scalar engine starts exp work ~5us in.
            early = [Unit(0, 0), Unit(1, 0), Unit(0, 1), Unit(1, 1), Unit(0, 2)]
            u0 = early[0]
            for th in range(4):
                qk_half(0, th)
                u0.emit_score(2 * th)
                u0.emit_score(2 * th + 1)

            # ---- phase W: weave V-proj + QK-jt1 between early-unit scores ----
            heavy = []
            vq = [lambda tg=tg: v_tile(tg) for tg in range(8)]
            jq = [lambda th=th: qk_half(1, th) for th in range(4)]
            order = [jq[0], vq[0], vq[1], jq[1], vq[2], vq[3], jq[2], vq[4], vq[5], jq[3], vq[6], vq[7]]
            scores = [(u, kg) for u in early[1:] for kg in range(8)]
            si = 0
            for i, hv in enumerate(order):
                # scores first in each slot to keep the Scalar engine fed
                n = (len(scores) * (i + 1)) // len(order) - si
                for _ in range(n):
                    u, kg = scores[si]
                    u.emit_score(kg)
                    si += 1
                hv()
            while si < len(scores):
                u, kg = scores[si]
                u.emit_score(kg)
                si += 1

            # ---- phase E: attention + out-projection pipeline ----
            # Scores of unit i interleave with PVs of earlier units (FIFO,
            # ~1.23 PV steps per score step so 104 score steps cover all
            # 128 PV steps). outproj(qb) fires as soon as its 4 units'
            # PVs (and normalize) are done.
            from collections import deque

            pv_queue = deque(early)
            units = {(u.h, u.qb): u for u in early}
            qb_done = [0, 0, 0, 0]
            state = {"drained": 0, "step": 0}

            def drain_one():
                if not pv_queue:
                    return False
                u = pv_queue[0]
                if u.n_pv >= len(u.es):
                    return False  # cannot run ahead of issued exps
                u.emit_pv()
                state["drained"] += 1
                if u.n_pv == 8:
                    pv_queue.popleft()
                    qb_done[u.qb] += 1
                    if qb_done[u.qb] == 4:
                        outproj(u.qb)
                return True

            rest = [(h, qb) for qb in range(4) for h in range(4)]
            rest = [hq for hq in rest if hq not in units]
            n_steps = len(rest) * 8  # 104
            for h, qb in rest:
                u = Unit(h, qb)
                units[(h, qb)] = u
                pv_queue.append(u)
                for kg in range(8):
                    u.emit_score(kg)
                    state["step"] += 1
                    target = (128 * state["step"]) // n_steps
                    while state["drained"] < target:
                        if not drain_one():
                            break
            while pv_queue:
                if not drain_one():
                    raise RuntimeError("pv pipeline wedged")

    nc.finalize()
    return nc


def _get_nc():
    global _CACHED_NC
    if _CACHED_NC is None:
        _CACHED_NC = _build()
    return _CACHED_NC


def make_in_maps(x, Wq, bq, Wk, bk, Wv, Wo):
    """Per-core input dicts (host-side sharding + layout + fp16 cast)."""
    xts = [
        np.ascontiguousarray(np.asarray(x[b], np.float32).T).astype(NPDT)
        for b in range(B)
    ]
    wq_f = np.asarray(Wq, np.float32) / 8.0
    wk_f = np.asarray(Wk, np.float32)
    wv_f = np.asarray(Wv, np.float32)
    wo_f = np.asarray(Wo, np.float32)
    bq_f = np.asarray(bq, np.float32) / 8.0
    bk_f = np.asarray(bk, np.float32)
    in_maps = []
    for c in range(NCORES):
        b, g = c // GROUPS, c % GROUPS
        js = slice(g * JW, (g + 1) * JW)
        in_maps.append(
            {
                "xt": xts[b],
                "wq": np.ascontiguousarray(wq_f[:, js]).astype(NPDT),
                "wk": np.ascontiguousarray(wk_f[:, js]).astype(NPDT),
                "wv": np.ascontiguousarray(wv_f[:, js]).astype(NPDT),
                "wo": np.ascontiguousarray(wo_f[js, :]).astype(NPDT),
                "bq": np.ascontiguousarray(bq_f[js]),
                "bk": np.ascontiguousarray(bk_f[js]),
            }
        )
    return in_maps


def combine(results, bias_row):
    """Sum per-core head-group partials (fp16) and add the host bias row."""
    out = np.zeros((B, T, C), np.float32)
    for c in range(NCORES):
        out[c // GROUPS] += results[c]["out"].astype(np.float32)
    out += bias_row
    return out


def kernel(x, Wq, bq, Wk, bk, Wv, bv, Wo, bo):
    nc = _get_nc()
    in_maps = make_in_maps(x, Wq, bq, Wk, bk, Wv, Wo)
    res = run_bass_kernel_spmd(nc, in_maps, core_ids=list(range(NCORES)))
    bias_row = (
        np.asarray(bv, np.float32) @ np.asarray(Wo, np.float32)
        + np.asarray(bo, np.float32)
    ).astype(np.float32)
    return combine(res.results, bias_row)


# revision 12
# speedup vs baseline: 1.0968x; 1.0968x over previous
"""Multi-head self-attention (B=2, T=2048, C=1024, H=16) on 8 NeuronCores.

Sharding: core c -> (batch b = c//4, head-group g = c%4); each core computes
4 heads' attention for one batch plus its slice of the QKV/out projections.
Per-core partial outputs (over head groups) are summed on the host.

Device-side layout is fully transposed (feature dim on partitions):
  xt [C, T] -> QT/KT [256, T] (j on partitions), V natural [T, 256],
  ST = K Qt (scores transposed, tk on partitions).
The stationary PV operand is V extended with 64 columns of ones, so the
yext accumulator's rows 64..127 all hold the softmax denominator.

v2 schedule (single fused pipeline, derived from the v1 trace):
 - xt DMA is tq-block-major so the first QK psum tile only waits ~1MB.
 - QK-jt0 runs first; then V-proj + QK-jt1 are woven between the
   scores+exp of three (head, qblock) attention units so the Scalar
   engine starts exp work ~40us earlier than v1.
 - Attention PV is software-pipelined at lag 2 behind scores/exp so the
   in-order PE never blocks on the exp->PV dependency (v1 lost ~130us
   of PE waits to this).
 - The out projection for query-block qb is issued right after the four
   units of that qb, overlapping its PSUM eviction and output DMA with
   the next qb's attention. Output is fp16 (host sums partials in fp32).
"""

import numpy as np

import concourse.bacc as bacc
import concourse.mybir as mybir
import concourse.tile as tile
from concourse.bass_utils import run_bass_kernel_spmd

B, T, C, H = 2, 2048, 1024, 16
HD = C // H  # 64
NCORES = 8
GROUPS = 4  # head groups (one per core within a batch)
HPG = H // GROUPS  # heads per group = 4
JW = HPG * HD  # per-core projection slice width = 256

F32 = mybir.dt.float32
F16 = mybir.dt.float16
MMDT = mybir.dt.float16
NPDT = np.float16

_CACHED_NC = None


def _build():
    nc = bacc.Bacc("TRN2", target_bir_lowering=False, num_devices=NCORES)

    xt = nc.dram_tensor("xt", [C, T], MMDT, kind="ExternalInput")
    wq = nc.dram_tensor("wq", [C, JW], MMDT, kind="ExternalInput")
    wk = nc.dram_tensor("wk", [C, JW], MMDT, kind="ExternalInput")
    wv = nc.dram_tensor("wv", [C, JW], MMDT, kind="ExternalInput")
    wo = nc.dram_tensor("wo", [JW, C], MMDT, kind="ExternalInput")
    bq = nc.dram_tensor("bq", [JW], F32, kind="ExternalInput")
    bk = nc.dram_tensor("bk", [JW], F32, kind="ExternalInput")
    out = nc.dram_tensor("out", [T, C], F16, kind="ExternalOutput")

    xt_ap = xt[:, :].rearrange("(cc p) t -> p cc t", p=128)  # [128, 8, T]
    wq_ap = wq[:, :].rearrange("(cc p) j -> p cc j", p=128)  # [128, 8, 256]
    wk_ap = wk[:, :].rearrange("(cc p) j -> p cc j", p=128)
    wv_ap = wv[:, :].rearrange("(cc p) j -> p cc j", p=128)
    wo_ap = wo[:, :].rearrange("(jt p) m -> p jt m", p=128)  # [128, 2, C]
    bq_ap = bq[:].rearrange("(jt p) -> p jt", p=128)  # [128, 2]
    bk_ap = bk[:].rearrange("(jt p) -> p jt", p=128)

    with tile.TileContext(nc) as tc:
        with (
            tc.tile_pool(name="big", bufs=1) as big,
            tc.tile_pool(name="es", bufs=46) as esp,
            tc.tile_pool(name="nrm", bufs=4) as nrm,
            tc.tile_pool(name="outp", bufs=3) as outp,
            tc.tile_pool(name="psA", bufs=3, space="PSUM") as psA,
            tc.tile_pool(name="psY", bufs=2, space="PSUM") as psY,
        ):
            # ---- persistent SBUF tensors ----
            xt_sb = big.tile([128, 8, T], MMDT)
            wq_sb = big.tile([128, 8, JW], MMDT)
            wk_sb = big.tile([128, 8, JW], MMDT)
            wv_sb = big.tile([128, 8, JW], MMDT)
            wo_sb = big.tile([128, 2, C], MMDT)
            qt_sb = big.tile([128, 2, T], MMDT)
            kt_sb = big.tile([128, 2, T], MMDT)
            yt_sb = big.tile([128, 2, T], MMDT)
            # V natural + 64 ones columns per head (denominator broadcast rows)
            v_sb = big.tile([128, 16, HPG, 128], MMDT)
            bq_sb = big.tile([128, 2], F32)
            bk_sb = big.tile([128, 2], F32)

            # ---- DMA: few, large dma_starts (each costs ~0.9us of Sync
            # queue issue time); xt is tq-block-major so the first QK tile
            # only waits for block 0 (~1MB).
            nc.sync.dma_start(out=wk_sb[:], in_=wk_ap)
            nc.sync.dma_start(out=xt_sb[:, :, 0:512], in_=xt_ap[:, :, 0:512])
            nc.sync.dma_start(out=wq_sb[:], in_=wq_ap)
            nc.sync.dma_start(out=bk_sb[:], in_=bk_ap)
            nc.sync.dma_start(out=bq_sb[:], in_=bq_ap)
            nc.vector.memset(v_sb[:, :, :, HD:128], 1.0)
            for tqh in range(1, 4):
                ts = slice(tqh * 512, (tqh + 1) * 512)
                nc.sync.dma_start(out=xt_sb[:, :, ts], in_=xt_ap[:, :, ts])
            nc.sync.dma_start(out=wv_sb[:], in_=wv_ap)
            nc.sync.dma_start(out=wo_sb[:], in_=wo_ap)

            # ---- emit helpers (each op is a closure; order = issue order) ----

            def qk_half(jt, th):
                """K then Q projection for 512-col tq block th (0..3), one
                [128,1024] psum tile: cols 0:512 = K (bank 1), 512:1024 = Q
                (bank 2). K is evicted first so scores can start early."""
                pkq = psA.tile([128, 1024], F32, tag="mm", name="pkq")
                xs = slice(th * 512, (th + 1) * 512)
                for cc in range(8):
                    nc.tensor.matmul(
                        pkq[:, 0:512],
                        wk_sb[:, cc, jt * 128 : (jt + 1) * 128],
                        xt_sb[:, cc, xs],
                        start=(cc == 0),
                        stop=(cc == 7),
                    )
                nc.vector.tensor_scalar_add(
                    out=kt_sb[:, jt, xs], in0=pkq[:, 0:512], scalar1=bk_sb[:, jt : jt + 1]
                )
                for cc in range(8):
                    nc.tensor.matmul(
                        pkq[:, 512:1024],
                        wq_sb[:, cc, jt * 128 : (jt + 1) * 128],
                        xt_sb[:, cc, xs],
                        start=(cc == 0),
                        stop=(cc == 7),
                    )
                nc.vector.tensor_scalar_add(
                    out=qt_sb[:, jt, xs], in0=pkq[:, 512:1024], scalar1=bq_sb[:, jt : jt + 1]
                )

            def v_tile(tg):
                """V projection for t-chunks 2*tg, 2*tg+1."""
                pv = psA.tile([128, 1024], F32, tag="mm", name="pv")
                for half in range(2):
                    tt = tg * 2 + half
                    for cc in range(8):
                        nc.tensor.matmul(
                            pv[:, half * 512 : half * 512 + JW],
                            xt_sb[:, cc, tt * 128 : (tt + 1) * 128],
                            wv_sb[:, cc, :],
                            start=(cc == 0),
                            stop=(cc == 7),
                        )
                pv3 = pv[:].rearrange("p (half j) -> p half j", half=2)
                nc.vector.tensor_copy(
                    out=v_sb[:, tg * 2 : tg * 2 + 2, :, 0:HD],
                    in_=pv3[:, :, 0:JW].rearrange("p half (h d) -> p half h d", h=HPG),
                )

            def score_step(h, qb, kg, es_list):
                """Scores + exp for kc pair (2kg, 2kg+1); appends the es tile."""
                jt, pb = h // 2, 64 * (h % 2)
                qs = slice(qb * 512, (qb + 1) * 512)
                st = psA.tile([128, 1024], F32, tag="mm", name="st")
                for half in range(2):
                    kc = kg * 2 + half
                    nc.tensor.matmul(
                        st[:, half * 512 : (half + 1) * 512],
                        kt_sb[pb : pb + HD, jt, kc * 128 : (kc + 1) * 128],
                        qt_sb[pb : pb + HD, jt, qs],
                        start=True,
                        stop=True,
                    )
                es = esp.tile([128, 1024], MMDT, tag="es", name="es")
                nc.scalar.activation(
                    out=es[:], in_=st[:], func=mybir.ActivationFunctionType.Exp
                )
                es_list.append(es)

            class Unit:
                """Attention unit (h, qb): 8 score/exp steps + 8 PV steps."""

                def __init__(self, h, qb):
                    self.h, self.qb = h, qb
                    self.es = []
                    self.yext = None
                    self.n_pv = 0

                def emit_score(self, kg):
                    score_step(self.h, self.qb, kg, self.es)

                def emit_pv(self):
                    kg = self.n_pv
                    if kg == 0:
                        self.yext = psY.tile([128, 512], F32, tag="yext", name="yext")
                    es = self.es[kg]
                    for half in range(2):
                        kc = kg * 2 + half
                        nc.tensor.matmul(
                            self.yext[:],
                            v_sb[:, kc, self.h, :],
                            es[:, half * 512 : (half + 1) * 512],
                            start=(kc == 0),
                            stop=(kc == 15),
                        )
                    self.n_pv += 1
                    if self.n_pv == 8:
                        self._normalize()

                def _normalize(self):
                    jt, pb = self.h // 2, 64 * (self.h % 2)
                    qs = slice(self.qb * 512, (self.qb + 1) * 512)
                    r32 = nrm.tile([HD, 512], F32, tag="r32", name="r32")
                    nc.vector.reciprocal(out=r32[:], in_=self.yext[HD:128, :])
                    nc.vector.tensor_mul(
                        out=yt_sb[pb : pb + HD, jt, qs],
                        in0=r32[:],
                        in1=self.yext[0:HD, :],
                    )

            def outproj_tt(tt):
                po = psA.tile([128, 1024], F32, tag="mm", name="po")
                for jt in range(2):
                    for mb in range(2):
                        nc.tensor.matmul(
                            po[:, mb * 512 : (mb + 1) * 512],
                            yt_sb[:, jt, tt * 128 : (tt + 1) * 128],
                            wo_sb[:, jt, mb * 512 : (mb + 1) * 512],
                            start=(jt == 0),
                            stop=(jt == 1),
                        )
                ob = outp.tile([128, 1024], F16, tag="ob", name="ob")
                nc.vector.tensor_copy(out=ob[:], in_=po[:])
                nc.sync.dma_start(out=out[tt * 128 : (tt + 1) * 128, :], in_=ob[:])

            # ---- phase A: QK jt0 woven with the first unit's scores so the
            # Scalar engine starts exp work ~5us in.
            early = [Unit(0, 0), Unit(1, 0), Unit(0, 1), Unit(1, 1), Unit(0, 2)]
            u0 = early[0]
            for th in range(4):
                qk_half(0, th)
                u0.emit_score(2 * th)
                u0.emit_score(2 * th + 1)

            # ---- phase W: weave V-proj + QK-jt1 between early-unit scores ----
            heavy = []
            vq = [lambda tg=tg: v_tile(tg) for tg in range(8)]
            jq = [lambda th=th: qk_half(1, th) for th in range(4)]
            order = [jq[0], vq[0], vq[1], jq[1], vq[2], vq[3], jq[2], vq[4], vq[5], jq[3], vq[6], vq[7]]
            scores = [(u, kg) for u in early[1:] for kg in range(8)]
            si = 0
            for i, hv in enumerate(order):
                # scores first in each slot to keep the Scalar engine fed
                n = (len(scores) * (i + 1)) // len(order) - si
                for _ in range(n):
                    u, kg = scores[si]
                    u.emit_score(kg)
                    si += 1
                hv()
            while si < len(scores):
                u, kg = scores[si]
                u.emit_score(kg)
                si += 1

            # ---- phase E: attention + out-projection pipeline ----
            # Scores of unit i interleave with PVs of earlier units (FIFO,
            # ~1.23 PV steps per score step so 104 score steps cover all
            # 128 PV steps). outproj(qb) fires as soon as its 4 units'
            # PVs (and normalize) are done.
            from collections import deque

            pv_queue = deque(early)
            po_queue = deque()
            units = {(u.h, u.qb): u for u in early}
            qb_done = [0, 0, 0, 0]
            state = {"drained": 0, "step": 0}

            def drain_one():
                if not pv_queue:
                    return False
                u = pv_queue[0]
                if u.n_pv >= len(u.es):
                    return False  # cannot run ahead of issued exps
                u.emit_pv()
                state["drained"] += 1
                if u.n_pv == 8:
                    pv_queue.popleft()
                    qb_done[u.qb] += 1
                    if qb_done[u.qb] == 4:
                        po_queue.extend(range(u.qb * 4, (u.qb + 1) * 4))
                return True

            rest = [(h, qb) for qb in range(4) for h in range(4)]
            rest = [hq for hq in rest if hq not in units]
            n_steps = len(rest) * 8  # 88
            for h, qb in rest:
                u = Unit(h, qb)
                units[(h, qb)] = u
                pv_queue.append(u)
                for kg in range(8):
                    u.emit_score(kg)
                    state["step"] += 1
                    target = (128 * state["step"]) // n_steps
                    while state["drained"] < target:
                        if not drain_one():
                            break
                # spread out-projection tiles between units
                if po_queue:
                    outproj_tt(po_queue.popleft())
                if len(po_queue) > 2:
                    outproj_tt(po_queue.popleft())
            while pv_queue:
                if not drain_one():
                    raise RuntimeError("pv pipeline wedged")
                if po_queue:
                    outproj_tt(po_queue.popleft())
            while po_queue:
                outproj_tt(po_queue.popleft())

    nc.finalize()
    return nc


def _get_nc():
    global _CACHED_NC
    if _CACHED_NC is None:
        _CACHED_NC = _build()
    return _CACHED_NC


def make_in_maps(x, Wq, bq, Wk, bk, Wv, Wo):
    """Per-core input dicts (host-side sharding + layout + fp16 cast)."""
    xts = [
        np.ascontiguousarray(np.asarray(x[b], np.float32).T).astype(NPDT)
        for b in range(B)
    ]
    wq_f = np.asarray(Wq, np.float32) / 8.0
    wk_f = np.asarray(Wk, np.float32)
    wv_f = np.asarray(Wv, np.float32)
    wo_f = np.asarray(Wo, np.float32)
    bq_f = np.asarray(bq, np.float32) / 8.0
    bk_f = np.asarray(bk, np.float32)
    in_maps = []
    for c in range(NCORES):
        b, g = c // GROUPS, c % GROUPS
        js = slice(g * JW, (g + 1) * JW)
        in_maps.append(
            {
                "xt": xts[b],
                "wq": np.ascontiguousarray(wq_f[:, js]).astype(NPDT),
                "wk": np.ascontiguousarray(wk_f[:, js]).astype(NPDT),
                "wv": np.ascontiguousarray(wv_f[:, js]).astype(NPDT),
                "wo": np.ascontiguousarray(wo_f[js, :]).astype(NPDT),
                "bq": np.ascontiguousarray(bq_f[js]),
                "bk": np.ascontiguousarray(bk_f[js]),
            }
        )
    return in_maps


def combine(results, bias_row):
    """Sum per-core head-group partials (fp16) and add the host bias row."""
    out = np.zeros((B, T, C), np.float32)
    for c in range(NCORES):
        out[c // GROUPS] += results[c]["out"].astype(np.float32)
    out += bias_row
    return out


def kernel(x, Wq, bq, Wk, bk, Wv, bv, Wo, bo):
    nc = _get_nc()
    in_maps = make_in_maps(x, Wq, bq, Wk, bk, Wv, Wo)
    res = run_bass_kernel_spmd(nc, in_maps, core_ids=list(range(NCORES)))
    bias_row = (
        np.asarray(bv, np.float32) @ np.asarray(Wo, np.float32)
        + np.asarray(bo, np.float32)
    ).astype(np.float32)
    return combine(res.results, bias_row)


# revision 18
# speedup vs baseline: 1.0994x; 1.0023x over previous
"""Multi-head self-attention (B=2, T=2048, C=1024, H=16) on 8 NeuronCores.

Sharding: core c -> (batch b = c//4, head-group g = c%4); each core computes
4 heads' attention for one batch plus its slice of the QKV/out projections.
Per-core partial outputs (over head groups) are summed on the host.

Device-side layout is fully transposed (feature dim on partitions):
  xt [C, T] -> QT/KT [256, T] (j on partitions), V natural [T, 256],
  ST = K Qt (scores transposed, tk on partitions).
The stationary PV operand is V extended with 64 columns of ones, so the
yext accumulator's rows 64..127 all hold the softmax denominator.

v2 schedule (single fused pipeline, derived from the v1 trace):
 - xt DMA is tq-block-major so the first QK psum tile only waits ~1MB.
 - QK-jt0 runs first; then V-proj + QK-jt1 are woven between the
   scores+exp of three (head, qblock) attention units so the Scalar
   engine starts exp work ~40us earlier than v1.
 - Attention PV is software-pipelined at lag 2 behind scores/exp so the
   in-order PE never blocks on the exp->PV dependency (v1 lost ~130us
   of PE waits to this).
 - The out projection for query-block qb is issued right after the four
   units of that qb, overlapping its PSUM eviction and output DMA with
   the next qb's attention. Output is fp16 (host sums partials in fp32).
"""

import numpy as np

import concourse.bacc as bacc
import concourse.mybir as mybir
import concourse.tile as tile
from concourse.bass_utils import run_bass_kernel_spmd

B, T, C, H = 2, 2048, 1024, 16
HD = C // H  # 64
NCORES = 8
GROUPS = 4  # head groups (one per core within a batch)
HPG = H // GROUPS  # heads per group = 4
JW = HPG * HD  # per-core projection slice width = 256

F32 = mybir.dt.float32
F16 = mybir.dt.float16
MMDT = mybir.dt.float16
NPDT = np.float16

_CACHED_NC = None


def _build():
    nc = bacc.Bacc("TRN2", target_bir_lowering=False, num_devices=NCORES)

    xt = nc.dram_tensor("xt", [C, T], MMDT, kind="ExternalInput")
    wq = nc.dram_tensor("wq", [C, JW], MMDT, kind="ExternalInput")
    wk = nc.dram_tensor("wk", [C, JW], MMDT, kind="ExternalInput")
    wv = nc.dram_tensor("wv", [C, JW], MMDT, kind="ExternalInput")
    wo = nc.dram_tensor("wo", [JW, C], MMDT, kind="ExternalInput")
    bq = nc.dram_tensor("bq", [JW], F32, kind="ExternalInput")
    bk = nc.dram_tensor("bk", [JW], F32, kind="ExternalInput")
    out = nc.dram_tensor("out", [T, C], F16, kind="ExternalOutput")

    xt_ap = xt[:, :].rearrange("(cc p) t -> p cc t", p=128)  # [128, 8, T]
    wq_ap = wq[:, :].rearrange("(cc p) j -> p cc j", p=128)  # [128, 8, 256]
    wk_ap = wk[:, :].rearrange("(cc p) j -> p cc j", p=128)
    wv_ap = wv[:, :].rearrange("(cc p) j -> p cc j", p=128)
    wo_ap = wo[:, :].rearrange("(jt p) m -> p jt m", p=128)  # [128, 2, C]
    bq_ap = bq[:].rearrange("(jt p) -> p jt", p=128)  # [128, 2]
    bk_ap = bk[:].rearrange("(jt p) -> p jt", p=128)

    with tile.TileContext(nc) as tc:
        with (
            tc.tile_pool(name="big", bufs=1) as big,
            tc.tile_pool(name="es", bufs=46) as esp,
            tc.tile_pool(name="nrm", bufs=4) as nrm,
            tc.tile_pool(name="outp", bufs=3) as outp,
            tc.tile_pool(name="psA", bufs=3, space="PSUM") as psA,
            tc.tile_pool(name="psY", bufs=2, space="PSUM") as psY,
        ):
            # ---- persistent SBUF tensors ----
            xt_sb = big.tile([128, 8, T], MMDT)
            wq_sb = big.tile([128, 8, JW], MMDT)
            wk_sb = big.tile([128, 8, JW], MMDT)
            wv_sb = big.tile([128, 8, JW], MMDT)
            wo_sb = big.tile([128, 2, C], MMDT)
            qt_sb = big.tile([128, 2, T], MMDT)
            kt_sb = big.tile([128, 2, T], MMDT)
            yt_sb = big.tile([128, 2, T], MMDT)
            # V natural + 64 ones columns per head (denominator broadcast rows)
            v_sb = big.tile([128, 16, HPG, 128], MMDT)
            bq_sb = big.tile([128, 2], F32)
            bk_sb = big.tile([128, 2], F32)

            # ---- DMA: few, large dma_starts (each costs ~0.9us of Sync
            # queue issue time); xt is tq-block-major so the first QK tile
            # only waits for block 0 (~1MB).
            nc.sync.dma_start(out=wk_sb[:], in_=wk_ap)
            nc.sync.dma_start(out=xt_sb[:, :, 0:512], in_=xt_ap[:, :, 0:512])
            nc.sync.dma_start(out=wq_sb[:], in_=wq_ap)
            nc.sync.dma_start(out=bk_sb[:], in_=bk_ap)
            nc.sync.dma_start(out=bq_sb[:], in_=bq_ap)
            for tqh in range(1, 4):
                ts = slice(tqh * 512, (tqh + 1) * 512)
                nc.sync.dma_start(out=xt_sb[:, :, ts], in_=xt_ap[:, :, ts])
            nc.sync.dma_start(out=wv_sb[:], in_=wv_ap)
            nc.sync.dma_start(out=wo_sb[:], in_=wo_ap)

            # ---- emit helpers (each op is a closure; order = issue order) ----

            def qk_half(jt, th):
                """K then Q projection for 512-col tq block th (0..3), one
                [128,1024] psum tile: cols 0:512 = K (bank 1), 512:1024 = Q
                (bank 2). K is evicted first so scores can start early."""
                pkq = psA.tile([128, 1024], F32, tag="mm", name="pkq")
                xs = slice(th * 512, (th + 1) * 512)
                for cc in range(8):
                    nc.tensor.matmul(
                        pkq[:, 0:512],
                        wk_sb[:, cc, jt * 128 : (jt + 1) * 128],
                        xt_sb[:, cc, xs],
                        start=(cc == 0),
                        stop=(cc == 7),
                    )
                nc.vector.tensor_scalar_add(
                    out=kt_sb[:, jt, xs], in0=pkq[:, 0:512], scalar1=bk_sb[:, jt : jt + 1]
                )
                for cc in range(8):
                    nc.tensor.matmul(
                        pkq[:, 512:1024],
                        wq_sb[:, cc, jt * 128 : (jt + 1) * 128],
                        xt_sb[:, cc, xs],
                        start=(cc == 0),
                        stop=(cc == 7),
                    )
                nc.vector.tensor_scalar_add(
                    out=qt_sb[:, jt, xs], in0=pkq[:, 512:1024], scalar1=bq_sb[:, jt : jt + 1]
                )

            def v_tile(tg):
                """V projection for t-chunks 2*tg, 2*tg+1."""
                pv = psA.tile([128, 1024], F32, tag="mm", name="pv")
                for half in range(2):
                    tt = tg * 2 + half
                    for cc in range(8):
                        nc.tensor.matmul(
                            pv[:, half * 512 : half * 512 + JW],
                            xt_sb[:, cc, tt * 128 : (tt + 1) * 128],
                            wv_sb[:, cc, :],
                            start=(cc == 0),
                            stop=(cc == 7),
                        )
                pv3 = pv[:].rearrange("p (half j) -> p half j", half=2)
                nc.vector.tensor_copy(
                    out=v_sb[:, tg * 2 : tg * 2 + 2, :, 0:HD],
                    in_=pv3[:, :, 0:JW].rearrange("p half (h d) -> p half h d", h=HPG),
                )

            def score_step(h, qb, kg, es_list):
                """Scores + exp for kc pair (2kg, 2kg+1); appends the es tile."""
                jt, pb = h // 2, 64 * (h % 2)
                qs = slice(qb * 512, (qb + 1) * 512)
                st = psA.tile([128, 1024], F32, tag="mm", name="st")
                for half in range(2):
                    kc = kg * 2 + half
                    nc.tensor.matmul(
                        st[:, half * 512 : (half + 1) * 512],
                        kt_sb[pb : pb + HD, jt, kc * 128 : (kc + 1) * 128],
                        qt_sb[pb : pb + HD, jt, qs],
                        start=True,
                        stop=True,
                    )
                es = esp.tile([128, 1024], MMDT, tag="es", name="es")
                nc.scalar.activation(
                    out=es[:], in_=st[:], func=mybir.ActivationFunctionType.Exp
                )
                es_list.append(es)

            class Unit:
                """Attention unit (h, qb): 8 score/exp steps + 8 PV steps."""

                def __init__(self, h, qb):
                    self.h, self.qb = h, qb
                    self.es = []
                    self.yext = None
                    self.n_pv = 0

                def emit_score(self, kg):
                    score_step(self.h, self.qb, kg, self.es)

                def emit_pv(self):
                    kg = self.n_pv
                    if kg == 0:
                        self.yext = psY.tile([128, 512], F32, tag="yext", name="yext")
                    es = self.es[kg]
                    for half in range(2):
                        kc = kg * 2 + half
                        nc.tensor.matmul(
                            self.yext[:],
                            v_sb[:, kc, self.h, :],
                            es[:, half * 512 : (half + 1) * 512],
                            start=(kc == 0),
                            stop=(kc == 15),
                        )
                    self.n_pv += 1
                    if self.n_pv == 8:
                        self._normalize()

                def _normalize(self):
                    jt, pb = self.h // 2, 64 * (self.h % 2)
                    qs = slice(self.qb * 512, (self.qb + 1) * 512)
                    r32 = nrm.tile([HD, 512], F32, tag="r32", name="r32")
                    nc.vector.reciprocal(out=r32[:], in_=self.yext[HD:128, :])
                    nc.vector.tensor_mul(
                        out=yt_sb[pb : pb + HD, jt, qs],
                        in0=r32[:],
                        in1=self.yext[0:HD, :],
                    )

            def outproj_tt(tt):
                po = psA.tile([128, 1024], F32, tag="mm", name="po")
                for jt in range(2):
                    for mb in range(2):
                        nc.tensor.matmul(
                            po[:, mb * 512 : (mb + 1) * 512],
                            yt_sb[:, jt, tt * 128 : (tt + 1) * 128],
                            wo_sb[:, jt, mb * 512 : (mb + 1) * 512],
                            start=(jt == 0),
                            stop=(jt == 1),
                        )
                ob = outp.tile([128, 1024], F16, tag="ob", name="ob")
                nc.vector.tensor_copy(out=ob[:], in_=po[:])
                nc.sync.dma_start(out=out[tt * 128 : (tt + 1) * 128, :], in_=ob[:])

            # ---- phase A: QK jt0 woven with the first two units' scores so
            # the Scalar engine starts exp work right away (also absorbs the
            # serial xt block DMA).
            early = [Unit(0, 0), Unit(1, 0), Unit(0, 1), Unit(1, 1), Unit(0, 2)]
            for th in range(4):
                qk_half(0, th)
                for u in early[:2]:
                    u.emit_score(2 * th)
                    u.emit_score(2 * th + 1)

            # ---- phase W: weave V-proj + QK-jt1 between early-unit scores ----
            heavy = []
            vq = [lambda tg=tg: v_tile(tg) for tg in range(8)]
            jq = [lambda th=th: qk_half(1, th) for th in range(4)]
            order = [jq[0], vq[0], vq[1], jq[1], vq[2], vq[3], jq[2], vq[4], vq[5], jq[3], vq[6], vq[7]]
            scores = [(u, kg) for u in early[2:] for kg in range(8)]
            si = 0
            for i, hv in enumerate(order):
                # scores first in each slot to keep the Scalar engine fed
                n = (len(scores) * (i + 1)) // len(order) - si
                for _ in range(n):
                    u, kg = scores[si]
                    u.emit_score(kg)
                    si += 1
                hv()
                if i == 2:
                    # denominator ones-columns; on DVE here so it neither
                    # delays the first kt eviction nor the first PV in E
                    nc.vector.memset(v_sb[:, :, :, HD:128], 1.0)
            while si < len(scores):
                u, kg = scores[si]
                u.emit_score(kg)
                si += 1

            # ---- phase E: attention + out-projection pipeline ----
            # Scores of unit i interleave with PVs of earlier units (FIFO,
            # ~1.23 PV steps per score step so 104 score steps cover all
            # 128 PV steps). outproj(qb) fires as soon as its 4 units'
            # PVs (and normalize) are done.
            from collections import deque

            pv_queue = deque(early)
            po_queue = deque()
            units = {(u.h, u.qb): u for u in early}
            qb_done = [0, 0, 0, 0]
            state = {"drained": 0, "step": 0}

            def drain_one():
                if not pv_queue:
                    return False
                u = pv_queue[0]
                if u.n_pv >= len(u.es):
                    return False  # cannot run ahead of issued exps
                u.emit_pv()
                state["drained"] += 1
                if u.n_pv == 8:
                    pv_queue.popleft()
                    qb_done[u.qb] += 1
                    if qb_done[u.qb] == 4:
                        po_queue.extend(range(u.qb * 4, (u.qb + 1) * 4))
                return True

            rest = [(h, qb) for qb in range(4) for h in range(4)]
            rest = [hq for hq in rest if hq not in units]
            n_steps = len(rest) * 8  # 88
            for h, qb in rest:
                u = Unit(h, qb)
                units[(h, qb)] = u
                pv_queue.append(u)
                for kg in range(8):
                    u.emit_score(kg)
                    state["step"] += 1
                    # initial burst of 16 (the two oldest units' exps are
                    # certainly done), then ~1.18/step so the PV lag shrinks
                    # from ~3 units to ~1 unit only at the very end.
                    target = 16 + (104 * state["step"]) // n_steps
                    while state["drained"] < target:
                        if not drain_one():
                            break
                # spread out-projection tiles between units
                if po_queue:
                    outproj_tt(po_queue.popleft())
                if len(po_queue) > 2:
                    outproj_tt(po_queue.popleft())
            while pv_queue:
                if not drain_one():
                    raise RuntimeError("pv pipeline wedged")
                if po_queue:
                    outproj_tt(po_queue.popleft())
            while po_queue:
                outproj_tt(po_queue.popleft())

    nc.finalize()
    return nc


def _get_nc():
    global _CACHED_NC
    if _CACHED_NC is None:
        _CACHED_NC = _build()
    return _CACHED_NC


def make_in_maps(x, Wq, bq, Wk, bk, Wv, Wo):
    """Per-core input dicts (host-side sharding + layout + fp16 cast)."""
    xts = [
        np.ascontiguousarray(np.asarray(x[b], np.float32).T).astype(NPDT)
        for b in range(B)
    ]
    wq_f = np.asarray(Wq, np.float32) / 8.0
    wk_f = np.asarray(Wk, np.float32)
    wv_f = np.asarray(Wv, np.float32)
    wo_f = np.asarray(Wo, np.float32)
    bq_f = np.asarray(bq, np.float32) / 8.0
    bk_f = np.asarray(bk, np.float32)
    in_maps = []
    for c in range(NCORES):
        b, g = c // GROUPS, c % GROUPS
        js = slice(g * JW, (g + 1) * JW)
        in_maps.append(
            {
                "xt": xts[b],
                "wq": np.ascontiguousarray(wq_f[:, js]).astype(NPDT),
                "wk": np.ascontiguousarray(wk_f[:, js]).astype(NPDT),
                "wv": np.ascontiguousarray(wv_f[:, js]).astype(NPDT),
                "wo": np.ascontiguousarray(wo_f[js, :]).astype(NPDT),
                "bq": np.ascontiguousarray(bq_f[js]),
                "bk": np.ascontiguousarray(bk_f[js]),
            }
        )
    return in_maps


def combine(results, bias_row):
    """Sum per-core head-group partials (fp16) and add the host bias row."""
    out = np.zeros((B, T, C), np.float32)
    for c in range(NCORES):
        out[c // GROUPS] += results[c]["out"].astype(np.float32)
    out += bias_row
    return out


def kernel(x, Wq, bq, Wk, bk, Wv, bv, Wo, bo):
    nc = _get_nc()
    in_maps = make_in_maps(x, Wq, bq, Wk, bk, Wv, Wo)
    res = run_bass_kernel_spmd(nc, in_maps, core_ids=list(range(NCORES)))
    bias_row = (
        np.asarray(bv, np.float32) @ np.asarray(Wo, np.float32)
        + np.asarray(bo, np.float32)
    ).astype(np.float32)
    return combine(res.results, bias_row)


# revision 20
# speedup vs baseline: 1.1086x; 1.0085x over previous
"""Multi-head self-attention (B=2, T=2048, C=1024, H=16) on 8 NeuronCores.

Sharding: core c -> (batch b = c//4, head-group g = c%4); each core computes
4 heads' attention for one batch plus its slice of the QKV/out projections.
Per-core partial outputs (over head groups) are summed on the host.

Device-side layout is fully transposed (feature dim on partitions):
  xt [C, T] -> QT/KT [256, T] (j on partitions), V natural [T, 256],
  ST = K Qt (scores transposed, tk on partitions).
The stationary PV operand is V extended with 64 columns of ones, so the
yext accumulator's rows 64..127 all hold the softmax denominator.

v2 schedule (single fused pipeline, derived from the v1 trace):
 - xt DMA is tq-block-major so the first QK psum tile only waits ~1MB.
 - QK-jt0 runs first; then V-proj + QK-jt1 are woven between the
   scores+exp of three (head, qblock) attention units so the Scalar
   engine starts exp work ~40us earlier than v1.
 - Attention PV is software-pipelined at lag 2 behind scores/exp so the
   in-order PE never blocks on the exp->PV dependency (v1 lost ~130us
   of PE waits to this).
 - The out projection for query-block qb is issued right after the four
   units of that qb, overlapping its PSUM eviction and output DMA with
   the next qb's attention. Output is fp16 (host sums partials in fp32).
"""

import numpy as np

import concourse.bacc as bacc
import concourse.mybir as mybir
import concourse.tile as tile
from concourse.bass_utils import run_bass_kernel_spmd

B, T, C, H = 2, 2048, 1024, 16
HD = C // H  # 64
NCORES = 8
GROUPS = 4  # head groups (one per core within a batch)
HPG = H // GROUPS  # heads per group = 4
JW = HPG * HD  # per-core projection slice width = 256

F32 = mybir.dt.float32
F16 = mybir.dt.float16
MMDT = mybir.dt.float16
NPDT = np.float16

_CACHED_NC = None


def _build():
    nc = bacc.Bacc("TRN2", target_bir_lowering=False, num_devices=NCORES)

    xt = nc.dram_tensor("xt", [C, T], MMDT, kind="ExternalInput")
    wq = nc.dram_tensor("wq", [C, JW], MMDT, kind="ExternalInput")
    wk = nc.dram_tensor("wk", [C, JW], MMDT, kind="ExternalInput")
    wv = nc.dram_tensor("wv", [C, JW], MMDT, kind="ExternalInput")
    wo = nc.dram_tensor("wo", [JW, C], MMDT, kind="ExternalInput")
    bq = nc.dram_tensor("bq", [JW], F32, kind="ExternalInput")
    bk = nc.dram_tensor("bk", [JW], F32, kind="ExternalInput")
    out = nc.dram_tensor("out", [T, C], F16, kind="ExternalOutput")

    xt_ap = xt[:, :].rearrange("(cc p) t -> p cc t", p=128)  # [128, 8, T]
    wq_ap = wq[:, :].rearrange("(cc p) j -> p cc j", p=128)  # [128, 8, 256]
    wk_ap = wk[:, :].rearrange("(cc p) j -> p cc j", p=128)
    wv_ap = wv[:, :].rearrange("(cc p) j -> p cc j", p=128)
    wo_ap = wo[:, :].rearrange("(jt p) m -> p jt m", p=128)  # [128, 2, C]
    bq_ap = bq[:].rearrange("(jt p) -> p jt", p=128)  # [128, 2]
    bk_ap = bk[:].rearrange("(jt p) -> p jt", p=128)

    with tile.TileContext(nc) as tc:
        with (
            tc.tile_pool(name="big", bufs=1) as big,
            tc.tile_pool(name="es", bufs=48) as esp,
            tc.tile_pool(name="nrm", bufs=4) as nrm,
            tc.tile_pool(name="outp", bufs=3) as outp,
            tc.tile_pool(name="psA", bufs=3, space="PSUM") as psA,
            tc.tile_pool(name="psY", bufs=2, space="PSUM") as psY,
        ):
            # ---- persistent SBUF tensors ----
            xt_sb = big.tile([128, 8, T], MMDT)
            wq_sb = big.tile([128, 8, JW], MMDT)
            wk_sb = big.tile([128, 8, JW], MMDT)
            wv_sb = big.tile([128, 8, JW], MMDT)
            wo_sb = big.tile([128, 2, C], MMDT)
            qt_sb = big.tile([128, 2, T], MMDT)
            kt_sb = big.tile([128, 2, T], MMDT)
            yt_sb = big.tile([128, 2, T], MMDT)
            # V natural + 64 ones columns per head (denominator broadcast rows)
            v_sb = big.tile([128, 16, HPG, 128], MMDT)
            bq_sb = big.tile([128, 2], F32)
            bk_sb = big.tile([128, 2], F32)

            # ---- DMA: few, large dma_starts (each costs ~0.9us of Sync
            # queue issue time); xt is tq-block-major so the first QK tile
            # only waits for block 0 (~1MB).
            nc.sync.dma_start(out=wk_sb[:], in_=wk_ap)
            nc.sync.dma_start(out=xt_sb[:, :, 0:512], in_=xt_ap[:, :, 0:512])
            nc.sync.dma_start(out=wq_sb[:], in_=wq_ap)
            nc.sync.dma_start(out=bk_sb[:], in_=bk_ap)
            nc.sync.dma_start(out=bq_sb[:], in_=bq_ap)
            for tqh in range(1, 4):
                ts = slice(tqh * 512, (tqh + 1) * 512)
                nc.sync.dma_start(out=xt_sb[:, :, ts], in_=xt_ap[:, :, ts])
            nc.sync.dma_start(out=wv_sb[:], in_=wv_ap)
            nc.sync.dma_start(out=wo_sb[:], in_=wo_ap)

            # ---- emit helpers (each op is a closure; order = issue order) ----

            def qk_half(jt, th):
                """K then Q projection for 512-col tq block th (0..3), one
                [128,1024] psum tile: cols 0:512 = K (bank 1), 512:1024 = Q
                (bank 2). K is evicted first so scores can start early."""
                pkq = psA.tile([128, 1024], F32, tag="mm", name="pkq")
                xs = slice(th * 512, (th + 1) * 512)
                for cc in range(8):
                    nc.tensor.matmul(
                        pkq[:, 0:512],
                        wk_sb[:, cc, jt * 128 : (jt + 1) * 128],
                        xt_sb[:, cc, xs],
                        start=(cc == 0),
                        stop=(cc == 7),
                    )
                nc.vector.tensor_scalar_add(
                    out=kt_sb[:, jt, xs], in0=pkq[:, 0:512], scalar1=bk_sb[:, jt : jt + 1]
                )
                for cc in range(8):
                    nc.tensor.matmul(
                        pkq[:, 512:1024],
                        wq_sb[:, cc, jt * 128 : (jt + 1) * 128],
                        xt_sb[:, cc, xs],
                        start=(cc == 0),
                        stop=(cc == 7),
                    )
                nc.vector.tensor_scalar_add(
                    out=qt_sb[:, jt, xs], in0=pkq[:, 512:1024], scalar1=bq_sb[:, jt : jt + 1]
                )

            def v_tile(tg):
                """V projection for t-chunks 2*tg, 2*tg+1."""
                pv = psA.tile([128, 1024], F32, tag="mm", name="pv")
                for half in range(2):
                    tt = tg * 2 + half
                    for cc in range(8):
                        nc.tensor.matmul(
                            pv[:, half * 512 : half * 512 + JW],
                            xt_sb[:, cc, tt * 128 : (tt + 1) * 128],
                            wv_sb[:, cc, :],
                            start=(cc == 0),
                            stop=(cc == 7),
                        )
                pv3 = pv[:].rearrange("p (half j) -> p half j", half=2)
                nc.vector.tensor_copy(
                    out=v_sb[:, tg * 2 : tg * 2 + 2, :, 0:HD],
                    in_=pv3[:, :, 0:JW].rearrange("p half (h d) -> p half h d", h=HPG),
                )

            def score_step(h, qb, kg, es_list):
                """Scores + exp for kc pair (2kg, 2kg+1); appends the es tile."""
                jt, pb = h // 2, 64 * (h % 2)
                qs = slice(qb * 512, (qb + 1) * 512)
                st = psA.tile([128, 1024], F32, tag="mm", name="st")
                for half in range(2):
                    kc = kg * 2 + half
                    nc.tensor.matmul(
                        st[:, half * 512 : (half + 1) * 512],
                        kt_sb[pb : pb + HD, jt, kc * 128 : (kc + 1) * 128],
                        qt_sb[pb : pb + HD, jt, qs],
                        start=True,
                        stop=True,
                    )
                es = esp.tile([128, 1024], MMDT, tag="es", name="es")
                nc.scalar.activation(
                    out=es[:], in_=st[:], func=mybir.ActivationFunctionType.Exp
                )
                es_list.append(es)

            class Unit:
                """Attention unit (h, qb): 8 score/exp steps + 8 PV steps."""

                def __init__(self, h, qb):
                    self.h, self.qb = h, qb
                    self.es = []
                    self.yext = None
                    self.n_pv = 0

                def emit_score(self, kg):
                    score_step(self.h, self.qb, kg, self.es)

                def emit_pv(self):
                    kg = self.n_pv
                    if kg == 0:
                        self.yext = psY.tile([128, 512], F32, tag="yext", name="yext")
                    es = self.es[kg]
                    for half in range(2):
                        kc = kg * 2 + half
                        nc.tensor.matmul(
                            self.yext[:],
                            v_sb[:, kc, self.h, :],
                            es[:, half * 512 : (half + 1) * 512],
                            start=(kc == 0),
                            stop=(kc == 15),
                        )
                    self.n_pv += 1
                    if self.n_pv == 8:
                        self._normalize()

                def _normalize(self):
                    jt, pb = self.h // 2, 64 * (self.h % 2)
                    qs = slice(self.qb * 512, (self.qb + 1) * 512)
                    r32 = nrm.tile([HD, 512], F32, tag="r32", name="r32")
                    nc.vector.reciprocal(out=r32[:], in_=self.yext[HD:128, :])
                    nc.vector.tensor_mul(
                        out=yt_sb[pb : pb + HD, jt, qs],
                        in0=r32[:],
                        in1=self.yext[0:HD, :],
                    )

            def outproj_tt(tt):
                po = psA.tile([128, 1024], F32, tag="mm", name="po")
                for jt in range(2):
                    for mb in range(2):
                        nc.tensor.matmul(
                            po[:, mb * 512 : (mb + 1) * 512],
                            yt_sb[:, jt, tt * 128 : (tt + 1) * 128],
                            wo_sb[:, jt, mb * 512 : (mb + 1) * 512],
                            start=(jt == 0),
                            stop=(jt == 1),
                        )
                ob = outp.tile([128, 1024], F16, tag="ob", name="ob")
                nc.vector.tensor_copy(out=ob[:], in_=po[:])
                nc.sync.dma_start(out=out[tt * 128 : (tt + 1) * 128, :], in_=ob[:])

            # ---- phase A: QK jt0 woven with the first two units' scores so
            # the Scalar engine starts exp work right away (also absorbs the
            # serial xt block DMA).
            early = [Unit(0, 0), Unit(1, 0), Unit(0, 1), Unit(1, 1), Unit(0, 2)]
            for th in range(4):
                qk_half(0, th)
                for u in early[:2]:
                    u.emit_score(2 * th)
                    u.emit_score(2 * th + 1)

            # ---- phase W: weave V-proj + QK-jt1 between early-unit scores ----
            heavy = []
            vq = [lambda tg=tg: v_tile(tg) for tg in range(8)]
            jq = [lambda th=th: qk_half(1, th) for th in range(4)]
            order = [jq[0], vq[0], vq[1], jq[1], vq[2], vq[3], jq[2], vq[4], vq[5], jq[3], vq[6], vq[7]]
            scores = [(u, kg) for u in early[2:] for kg in range(8)]
            si = 0
            for i, hv in enumerate(order):
                # scores first in each slot to keep the Scalar engine fed
                n = (len(scores) * (i + 1)) // len(order) - si
                for _ in range(n):
                    u, kg = scores[si]
                    u.emit_score(kg)
                    si += 1
                hv()
                if i == 2:
                    # denominator ones-columns; on DVE here so it neither
                    # delays the first kt eviction nor the first PV in E
                    nc.vector.memset(v_sb[:, :, :, HD:128], 1.0)
            while si < len(scores):
                u, kg = scores[si]
                u.emit_score(kg)
                si += 1

            # ---- phase E: attention + out-projection pipeline ----
            # Scores of unit i interleave with PVs of earlier units (FIFO,
            # ~1.23 PV steps per score step so 104 score steps cover all
            # 128 PV steps). outproj(qb) fires as soon as its 4 units'
            # PVs (and normalize) are done.
            from collections import deque

            pv_queue = deque(early)
            po_queue = deque()
            units = {(u.h, u.qb): u for u in early}
            qb_done = [0, 0, 0, 0]
            state = {"drained": 0, "step": 0}

            def drain_one():
                if not pv_queue:
                    return False
                u = pv_queue[0]
                if u.n_pv >= len(u.es):
                    return False  # cannot run ahead of issued exps
                u.emit_pv()
                state["drained"] += 1
                if u.n_pv == 8:
                    pv_queue.popleft()
                    qb_done[u.qb] += 1
                    if qb_done[u.qb] == 4:
                        po_queue.extend(range(u.qb * 4, (u.qb + 1) * 4))
                return True

            rest = [(h, qb) for qb in range(4) for h in range(4)]
            rest = [hq for hq in rest if hq not in units]
            n_steps = len(rest) * 8  # 88
            for h, qb in rest:
                u = Unit(h, qb)
                units[(h, qb)] = u
                pv_queue.append(u)
                for kg in range(8):
                    u.emit_score(kg)
                    state["step"] += 1
                    # constant emission lag of 32 PV-steps (~4 units): the
                    # Scalar engine's runtime backlog grows to ~26 steps by
                    # the end, and the PE must never reach a PV whose exp
                    # hasn't run. The leftover 32 steps drain in the tail
                    # while the Scalar engine chews its final backlog.
                    target = min(128, state["step"] + 8)
                    while state["drained"] < target:
                        if not drain_one():
                            break
                # spread out-projection tiles between units
                if po_queue:
                    outproj_tt(po_queue.popleft())
                if len(po_queue) > 2:
                    outproj_tt(po_queue.popleft())
            while pv_queue:
                if not drain_one():
                    raise RuntimeError("pv pipeline wedged")
                if po_queue:
                    outproj_tt(po_queue.popleft())
            while po_queue:
                outproj_tt(po_queue.popleft())

    nc.finalize()
    return nc


def _get_nc():
    global _CACHED_NC
    if _CACHED_NC is None:
        _CACHED_NC = _build()
    return _CACHED_NC


def make_in_maps(x, Wq, bq, Wk, bk, Wv, Wo):
    """Per-core input dicts (host-side sharding + layout + fp16 cast)."""
    xts = [
        np.ascontiguousarray(np.asarray(x[b], np.float32).T).astype(NPDT)
        for b in range(B)
    ]
    wq_f = np.asarray(Wq, np.float32) / 8.0
    wk_f = np.asarray(Wk, np.float32)
    wv_f = np.asarray(Wv, np.float32)
    wo_f = np.asarray(Wo, np.float32)
    bq_f = np.asarray(bq, np.float32) / 8.0
    bk_f = np.asarray(bk, np.float32)
    in_maps = []
    for c in range(NCORES):
        b, g = c // GROUPS, c % GROUPS
        js = slice(g * JW, (g + 1) * JW)
        in_maps.append(
            {
                "xt": xts[b],
                "wq": np.ascontiguousarray(wq_f[:, js]).astype(NPDT),
                "wk": np.ascontiguousarray(wk_f[:, js]).astype(NPDT),
                "wv": np.ascontiguousarray(wv_f[:, js]).astype(NPDT),
                "wo": np.ascontiguousarray(wo_f[js, :]).astype(NPDT),
                "bq": np.ascontiguousarray(bq_f[js]),
                "bk": np.ascontiguousarray(bk_f[js]),
            }
        )
    return in_maps


def combine(results, bias_row):
    """Sum per-core head-group partials (fp16) and add the host bias row."""
    out = np.zeros((B, T, C), np.float32)
    for c in range(NCORES):
        out[c // GROUPS] += results[c]["out"].astype(np.float32)
    out += bias_row
    return out


def kernel(x, Wq, bq, Wk, bk, Wv, bv, Wo, bo):
    nc = _get_nc()
    in_maps = make_in_maps(x, Wq, bq, Wk, bk, Wv, Wo)
    res = run_bass_kernel_spmd(nc, in_maps, core_ids=list(range(NCORES)))
    bias_row = (
        np.asarray(bv, np.float32) @ np.asarray(Wo, np.float32)
        + np.asarray(bo, np.float32)
    ).astype(np.float32)
    return combine(res.results, bias_row)
